# revision 1
# baseline (speedup 1.0000x reference)
"""LIF v3: 2 custom DVE ops/step + ScalarE sign + GpSimd vp (wbar=0.9w state) — 2 DVE ops/step + ScalarE sign + GpSimd vp.

Recurrence rewritten with carried states u_t (pre-threshold potential) and
w_t (adaptation):
    s_t    = 1[u_t > 0.5]                     (spike, output)
    w_{t+1} = 0.9*w_t + 0.05*(u_t + s_t)       custom W_OP   (VectorE)
    vp_{t+1} = x_{t+1} - 0.9*w_t               STT           (GpSimd)
    u_{t+1} = 0.45*u_t - 0.3*s_t + vp_{t+1}    custom U_OP   (VectorE)
Output via ScalarE: sgn_t = Sign(u_t - 0.5) -> int8 in {-1,0,1}; host maps
(sgn > 0) -> {0,1} f32.  Derivation: u = 0.5*mem + x - w with
mem' = u - 0.5 s, w' = 0.9 w + 0.05 (u + s).
"""

import numpy as np

import concourse.bass as bass
import concourse.bacc as bacc
import concourse.mybir as mybir
import concourse.tile as tile
from concourse.bass_utils import run_bass_kernel_spmd

import concourse.dve_ops as dops
from concourse.dve_ops import DveOp
from concourse.dve_spec import Spec, Src0, Src1, C0, C1, C2, lower
from concourse.dve_ops import has_src1
from concourse.dve_uop import DveOpSpec

B, N, T = 64, 8192, 100
N_CORES = 8
P = 128
CH = 20

F32 = mybir.dt.float32
I8 = mybir.dt.int8
Alu = mybir.AluOpType
Act = mybir.ActivationFunctionType


def _register(name, spec):
    for o in dops.OPS:
        if o.name == name:
            return o
    opcode = dops._CUSTOM_DVE_ROW_BASE + len(dops.OPS)
    assert opcode < 0x20
    shas = {}
    for ver in ("v3", "v4"):
        dspec = DveOpSpec(
            name=name, opcode=opcode, uops=lower(spec, ver=ver),
            rd1_en=has_src1(spec),
        )
        shas[ver] = dspec.sha(ver)
    op = DveOp(name, spec, subdim=False, uops_sha=shas)
    dops.OPS.append(op)
    dops._SUB_OPCODE_FOR_NAME[name] = opcode
    dops.CUSTOM_DVE_SPECS[name] = spec
    return op


# w' = s0*in1 + s1*(in0 + (in0 > imm2))
LIF_W = _register(
    "LIF_W_ANT",
    Spec(
        body=Src1 * C0 + (Src0 + (Src0 > C2)) * C1,
        reference=lambda in0, in1, s0, s1, imm2: in1 * s0
        + (in0 + (in0 > imm2).astype(np.float32)) * s1,
    ),
)

# u' = s0*in0 - s1*(in0 > imm2) + in1
LIF_U = _register(
    "LIF_U_ANT",
    Spec(
        body=Src0 * C0 - (Src0 > C2) * C1 + Src1,
        reference=lambda in0, in1, s0, s1, imm2: in0 * s0
        - (in0 > imm2).astype(np.float32) * s1
        + in1,
    ),
)


def build_nc(T_: int, P_: int, F_: int, ch: int = CH):
    nc = bacc.Bacc("TRN2", target_bir_lowering=False, debug=False)
    E = P_ * F_
    n_ch = (T_ + ch - 1) // ch
    x_d = nc.dram_tensor("x", [T_, E], F32, kind="ExternalInput").ap()
    s_d = nc.dram_tensor("s", [T_, E], I8, kind="ExternalOutput").ap()

    with tile.TileContext(nc) as tc:
        with (
            tc.tile_pool(name="xp", bufs=2) as xp,
            tc.tile_pool(name="sp", bufs=2) as sp,
            tc.tile_pool(name="st", bufs=2) as st,
            tc.tile_pool(name="zp", bufs=1) as zp,
        ):
            def chunk_steps(k):
                return min(ch, T_ - k * ch)

            def load_chunk(k):
                n_t = chunk_steps(k)
                xt = xp.tile([P_, ch * F_], F32, tag="x")
                src = x_d[k * ch:k * ch + n_t].rearrange(
                    "t (p f) -> p t f", p=P_
                )
                nc.sync.dma_start(
                    xt[:].rearrange("p (t f) -> p t f", t=ch)[:, :n_t], src
                )
                return xt

            def x_slice(t):
                k, tl = divmod(t, ch)
                xc = x_chunk if k == cur_k else x_next_chunk
                return xc[:, tl * F_:(tl + 1) * F_]

            x_chunk = load_chunk(0)
            cur_k = 0
            s_chunk = sp.tile([P_, ch * F_], I8, tag="s")
            w_zero = zp.tile([P_, F_], F32, tag="wz")
            nc.gpsimd.memset(w_zero[:], 0.0)
            bias_m05 = zp.tile([P_, 1], F32, tag="b05")
            nc.gpsimd.memset(bias_m05[:], -0.5)

            u_prev = None
            w_prev = w_zero
            for t in range(T_):
                k, tl = divmod(t, ch)
                if tl == 0 and k + 1 < n_ch:
                    x_next_chunk = load_chunk(k + 1)

                u = u_prev if t > 0 else x_slice(0)

                # spike output: Sign(u - 0.5) -> int8 {-1,0,1}
                sg = s_chunk[:, tl * F_:(tl + 1) * F_]
                nc.scalar.activation(sg[:], u[:], Act.Sign, bias=bias_m05[:], scale=1.0)

                if t + 1 < T_:
                    w_new = st.tile([P_, F_], F32, tag="w")
                    nc.vector._custom_dve(
                        LIF_W, out=w_new[:], in0=u[:], in1=w_prev[:],
                        s0=0.9, s1=0.045, imm2=0.5,
                    )
                    # vp_{t+1} = x_{t+1} - 0.9*w_t   (GpSimd; t=0: w_0=0)
                    if t == 0:
                        vp = x_slice(1)
                    else:
                        vp = st.tile([P_, F_], F32, tag="vp")
                        nc.gpsimd.tensor_tensor(
                            vp[:], x_slice(t + 1)[:], w_prev[:], op=Alu.subtract
                        )
                    u_new = st.tile([P_, F_], F32, tag="u")
                    nc.vector._custom_dve(
                        LIF_U, out=u_new[:], in0=u[:], in1=vp[:],
                        s0=0.45, s1=0.3, imm2=0.5,
                    )
                    u_prev, w_prev = u_new, w_new

                if tl == chunk_steps(k) - 1:
                    n_t = chunk_steps(k)
                    dst = s_d[k * ch:k * ch + n_t].rearrange(
                        "t (p f) -> p t f", p=P_
                    )
                    nc.sync.dma_start(
                        dst,
                        s_chunk[:].rearrange("p (t f) -> p t f", t=ch)[:, :n_t],
                    )
                    if t + 1 < T_:
                        x_chunk = x_next_chunk
                        cur_k = k + 1
                        s_chunk = sp.tile([P_, ch * F_], I8, tag="s")
    nc.compile()
    return nc


def postprocess_core(core_result: dict) -> np.ndarray:
    return (core_result["s"].T > 0).astype(np.float32)


def _run(x: np.ndarray, trace: bool = False):
    x = np.asarray(x)
    b, n, t_ = x.shape
    e_tot = b * n
    e = e_tot // N_CORES
    f = e // P
    nc = build_nc(t_, P, f)
    xf = x.reshape(e_tot, t_)
    in_maps = [
        {"x": np.ascontiguousarray(xf[c * e:(c + 1) * e].T)}
        for c in range(N_CORES)
    ]
    bkr = run_bass_kernel_spmd(nc, in_maps, list(range(N_CORES)), trace=False)
    res = bkr.results
    out = np.concatenate([postprocess_core(res[c]) for c in range(N_CORES)], axis=0)
    return np.ascontiguousarray(out.reshape(b, n, t_)).astype(np.float32), bkr


def kernel(x: np.ndarray) -> np.ndarray:
    return _run(x)[0]

